# revision 1
# baseline (speedup 1.0000x reference)
"""Trainium2 Bass kernel for the GRUCell-variant problem.

  z = sigmoid(x@Wiz + h@Uhz + bz)
  r = sigmoid(x@Wir + h@Uhr + br)
  n = sigmoid(x@Win + (r*h)@Uhn + bn)
  out = (1-z)*h + z*n

Full shapes: x,h [8192,1024]; W*,U* [1024,1024]; b* [1024].
Sharding: data-parallel over batch across 8 NeuronCores (1024 rows each);
weights replicated; no collectives.

Per-core dataflow (all matmuls in fp32r, 1 cyc/row on the PE at N=512):
  1. DMA x,h batch-strips [128,1024]; PE-transpose into feature-major
     xT,hT [128(d%128), 8(d//128), 1024(b)] resident in SBUF.
  2. Phase R (weights-stationary, feature-major out):
     psum_r[128h,512b] = sum_d Wir[d,hs].T@xT + Uhr[d,hs].T@hT (one PSUM
     accumulation group); r = ACT-sigmoid(psum + br) with per-partition bias.
  3. rh = r * hT elementwise on DVE (feature-major, no transpose needed).
  4. Phase ZN: psum_z = sum_d Wiz.T@xT + Uhz.T@hT; psum_n = sum_d Win.T@xT
     + Uhn.T@rh; z,n = ACT-sigmoid(+bias); out_fm = hT + z*(n-hT) on DVE.
  5. PE-transpose out_fm back to batch-major, contiguous DMA store.
"""

import sys

if "/opt/trn_rl_repo" not in sys.path:
    sys.path.insert(0, "/opt/trn_rl_repo")

import numpy as np

P = 128
B_FULL = 8192
D = 1024  # d_in == d_h == 1024
N_CORES = 8
B_CORE = B_FULL // N_CORES  # 1024
NS = D // P  # 8 strips of 128 along any 1024 dim
BH = 512  # moving free-dim per matmul (fp32 limit / one PSUM bank)
NBH = B_CORE // BH  # 2 batch halves

_NC_CACHE = {}


def _build_bass():
    import concourse.mybir as mybir
    import concourse.tile as tile
    from concourse import bacc
    from concourse.masks import make_identity

    F32 = mybir.dt.float32
    F32R = mybir.dt.float32r
    SIG = mybir.ActivationFunctionType.Sigmoid

    nc = bacc.Bacc(None, target_bir_lowering=False)

    x = nc.dram_tensor("x", [B_CORE, D], F32, kind="ExternalInput")
    h = nc.dram_tensor("h", [B_CORE, D], F32, kind="ExternalInput")
    wts = {
        name: nc.dram_tensor(name, [D, D], F32, kind="ExternalInput")
        for name in ("Wiz", "Uhz", "Wir", "Uhr", "Win", "Uhn")
    }
    biases = {
        name: nc.dram_tensor(name, [D], F32, kind="ExternalInput")
        for name in ("bz", "br", "bn")
    }
    out = nc.dram_tensor("out", [B_CORE, D], F32, kind="ExternalOutput")

    with tile.TileContext(nc) as tc:
        with (
            tc.tile_pool(name="big", bufs=1) as big,
            tc.tile_pool(name="nat", bufs=3) as natp,
            tc.tile_pool(name="wp", bufs=8) as wp,
            tc.tile_pool(name="tmp", bufs=6) as tmpp,
            tc.tile_pool(name="psmm", bufs=5, space="PSUM") as psmm,
            tc.tile_pool(name="pstr", bufs=3, space="PSUM") as pstr,
        ):
            ident = big.tile([P, P], F32, tag="ident")
            make_identity(nc, ident)

            # Resident feature-major activations: [p, o, b] = val[b, o*128+p]
            xT = big.tile([P, NS, B_CORE], F32R, tag="xT")
            hT = big.tile([P, NS, B_CORE], F32R, tag="hT")
            rh = big.tile([P, NS, B_CORE], F32R, tag="rh")
            out_bm = big.tile([P, NS, D], F32, tag="out_bm")

            # ---- biases -> per-partition layout [128, NS] via PE transpose
            bias_t = {}
            for name in ("bz", "br", "bn"):
                bnat = natp.tile([NS, P], F32, tag="bnat")
                nc.sync.dma_start(
                    out=bnat, in_=biases[name].ap().rearrange("(s p) -> s p", p=P)
                )
                psb = pstr.tile([P, NS], F32, tag="tr")
                nc.tensor.transpose(psb, bnat, ident[0:NS, 0:NS])
                bt = big.tile([P, NS], F32, tag=f"{name}_t")
                nc.vector.tensor_copy(out=bt, in_=psb)
                bias_t[name] = bt

            # ---- input transposes: x,h -> xT,hT
            for src, dst in ((x, xT), (h, hT)):
                for s in range(NS):  # batch strip
                    nat = natp.tile([P, D], F32, tag="nat")
                    nc.sync.dma_start(out=nat, in_=src.ap()[s * P:(s + 1) * P, :])
                    for o in range(NS):  # feature strip
                        pst = pstr.tile([P, P], F32, tag="tr")
                        nc.tensor.transpose(pst, nat[:, o * P:(o + 1) * P], ident)
                        nc.vector.tensor_copy(
                            out=dst[:, o, s * P:(s + 1) * P], in_=pst
                        )

            def load_wcol(wname, hs):
                """Load W[:, hs*128:(hs+1)*128] as [128(d%128), NS(d//128), 128(h)]."""
                wt = wp.tile([P, NS, P], F32R, tag="w")
                nc.sync.dma_start(
                    out=wt,
                    in_=wts[wname].ap().rearrange("(o p) n -> p o n", p=P)[
                        :, :, hs * P:(hs + 1) * P
                    ].bitcast(F32R),
                )
                return wt

            # ---- phase R: r (feature-major), then rh = r * hT
            for hs in range(NS):
                wW = load_wcol("Wir", hs)
                wU = load_wcol("Uhr", hs)
                for bh in range(NBH):
                    bs = slice(bh * BH, (bh + 1) * BH)
                    ps = psmm.tile([P, BH], F32, tag="mm")
                    for o in range(NS):
                        nc.tensor.matmul(
                            ps, wW[:, o, :], xT[:, o, bs],
                            start=(o == 0), stop=False,
                        )
                    for o in range(NS):
                        nc.tensor.matmul(
                            ps, wU[:, o, :], hT[:, o, bs],
                            start=False, stop=(o == NS - 1),
                        )
                    # r tile -> rh buffer (rh = sigmoid(ps + br) * hT below)
                    nc.scalar.activation(
                        rh[:, hs, bs], ps, SIG, bias=bias_t["br"][:, hs:hs + 1]
                    )
                nc.vector.tensor_mul(rh[:, hs, :], rh[:, hs, :], hT[:, hs, :])

            # ---- phase ZN + final combine + output transpose
            # bh outer: out_bm strips for a batch-half complete while the
            # other half computes, overlapping stores with matmuls (weights
            # are re-streamed per bh; DMA has headroom).
            for bh in range(NBH):
                bs = slice(bh * BH, (bh + 1) * BH)
                for hs in range(NS):
                    wZi = load_wcol("Wiz", hs)
                    wZu = load_wcol("Uhz", hs)
                    wNi = load_wcol("Win", hs)
                    wNu = load_wcol("Uhn", hs)
                    ps_z = psmm.tile([P, BH], F32, tag="mm")
                    for o in range(NS):
                        nc.tensor.matmul(
                            ps_z, wZi[:, o, :], xT[:, o, bs],
                            start=(o == 0), stop=False,
                        )
                    for o in range(NS):
                        nc.tensor.matmul(
                            ps_z, wZu[:, o, :], hT[:, o, bs],
                            start=False, stop=(o == NS - 1),
                        )
                    ps_n = psmm.tile([P, BH], F32, tag="mm")
                    for o in range(NS):
                        nc.tensor.matmul(
                            ps_n, wNi[:, o, :], xT[:, o, bs],
                            start=(o == 0), stop=False,
                        )
                    for o in range(NS):
                        nc.tensor.matmul(
                            ps_n, wNu[:, o, :], rh[:, o, bs],
                            start=False, stop=(o == NS - 1),
                        )
                    z_t = tmpp.tile([P, BH], F32, tag="gt")
                    nc.scalar.activation(
                        z_t, ps_z, SIG, bias=bias_t["bz"][:, hs:hs + 1]
                    )
                    n_t = tmpp.tile([P, BH], F32, tag="gt")
                    nc.scalar.activation(
                        n_t, ps_n, SIG, bias=bias_t["bn"][:, hs:hs + 1]
                    )
                    # out_fm = z*(n - h~); exact h added batch-major at store
                    d_t = tmpp.tile([P, BH], F32, tag="gt")
                    nc.vector.tensor_sub(d_t, n_t, hT[:, hs, bs].bitcast(F32))
                    nc.vector.tensor_mul(d_t, d_t, z_t)
                    # transpose back to batch-major
                    for q in range(BH // P):
                        pst = pstr.tile([P, P], F32, tag="tr")
                        nc.tensor.transpose(
                            pst, d_t[:, q * P:(q + 1) * P], ident
                        )
                        s = bh * (BH // P) + q  # batch strip index
                        nc.vector.tensor_copy(
                            out=out_bm[:, s, hs * P:(hs + 1) * P], in_=pst
                        )

                # store this batch-half: out = z*(n-h~) + h (exact h)
                for q in range(BH // P):
                    s = bh * (BH // P) + q
                    h_nat = natp.tile([P, D], F32, tag="nat")
                    nc.sync.dma_start(out=h_nat, in_=h.ap()[s * P:(s + 1) * P, :])
                    nc.vector.tensor_add(out_bm[:, s, :], out_bm[:, s, :], h_nat)
                    nc.sync.dma_start(
                        out=out.ap()[s * P:(s + 1) * P, :], in_=out_bm[:, s, :]
                    )

    nc.compile()
    return nc


def _get_nc():
    if "nc" not in _NC_CACHE:
        _NC_CACHE["nc"] = _build_bass()
    return _NC_CACHE["nc"]


def kernel(**inputs):
    from concourse.bass_utils import run_bass_kernel_spmd

    nc = _get_nc()
    full = {k: np.ascontiguousarray(np.asarray(v, dtype=np.float32))
            for k, v in inputs.items()}
    shared = {k: full[k] for k in
              ("Wiz", "Uhz", "bz", "Wir", "Uhr", "br", "Win", "Uhn", "bn")}
    in_maps = []
    for c in range(N_CORES):
        sl = slice(c * B_CORE, (c + 1) * B_CORE)
        m = {"x": full["x"][sl], "h": full["h"][sl]}
        m.update(shared)
        in_maps.append(m)
    res = run_bass_kernel_spmd(nc, in_maps, list(range(N_CORES)))
    return np.concatenate([res.results[c]["out"] for c in range(N_CORES)], axis=0)

